# revision 2
# baseline (speedup 1.0000x reference)
"""Trainium kernel for nn_BaseNeighborNN (B=16, N=1024, K=20).

Sharding: data-parallel over the batch dimension across 8 NeuronCores
(2 snapshots per core); weights replicated. Each core computes all-pairs
periodic distances, top-K neighbor selection, the 78 geometric pair
features, the MLP and the neighbor max-pool locally. Host code only
shards inputs / concatenates outputs.

The device attempt runs in a forked subprocess: the neuron toolchain
aborts the process on an unsupported-graph compile error, and the
subprocess boundary turns that into a recoverable failure (host-numpy
fallback keeps the contract).
"""
import os
import subprocess
import sys
import tempfile

import numpy as np

B, N, K = 16, 1024, 20
BOX = 10.0
NCORES = 8
BPC = B // NCORES

_DEVICE_WORKER = r'''
import sys, time
import numpy as np

inp_path, out_path = sys.argv[1], sys.argv[2]
data = np.load(inp_path)
position = data["position"]; orientation_R = data["orientation_R"]
w1 = data["w1"]; b1 = data["b1"]; w2 = data["w2"]; b2 = data["b2"]
w3 = data["w3"]; b3 = data["b3"]

B, N, K = 16, 1024, 20
BOX = 10.0
NCORES = 8
BPC = B // NCORES

import jax, jax.numpy as jnp

devs = [d for d in jax.devices() if d.platform != "cpu"][:NCORES]
assert len(devs) == NCORES, f"need {NCORES} neuron cores, got {len(devs)}"


def per_core(pos, oR, w1, b1, w2, b2, w3, b3):
    dr = pos[:, :, None, :] - pos[:, None, :, :]
    dr = dr - BOX * jnp.round(dr * (1.0 / BOX))
    r2 = jnp.sum(dr * dr, axis=-1)
    r2 = r2 + jnp.eye(N, dtype=r2.dtype) * 1e9
    neg_r2, idx = jax.lax.top_k(-r2, K)
    NN_R = jnp.sqrt(-neg_r2)[..., None]
    NN_dr = jnp.take_along_axis(dr, idx[..., None], axis=2)
    u = NN_dr / NN_R
    inv_r = 1.0 / NN_R
    bidx = jnp.arange(BPC)[:, None, None]
    Rn = oR[bidx, idx]
    Rp = oR
    dot = jnp.einsum("bnij,bnklj->bnkil", Rp, Rn)
    elem = Rp[:, :, None, :, None, :] * Rn[:, :, :, None, :, :]
    elem_norm = jnp.sqrt(jnp.sum(elem * elem, axis=-1))
    cross = jnp.cross(Rp[:, :, None, :, :], Rn)
    cross_norm = jnp.sqrt(jnp.sum(cross * cross, axis=-1))
    rel = jnp.einsum("bnji,bnkjl->bnkil", Rp, Rn)
    rbf_p = jnp.exp(-jnp.einsum("bnkd,bnid->bnki", u, Rp) ** 2)
    rbf_n = jnp.exp(-jnp.einsum("bnkd,bnkid->bnki", u, Rn) ** 2)
    tr = rel[..., 0, 0] + rel[..., 1, 1] + rel[..., 2, 2]
    ang = jnp.arccos(jnp.clip((tr - 1.0) * 0.5, -1.0 + 1e-6, 1.0 - 1e-6))[..., None]
    f = jnp.concatenate(
        [NN_R, inv_r, u,
         dot.reshape(BPC, N, K, 9),
         elem.reshape(BPC, N, K, 27),
         elem_norm.reshape(BPC, N, K, 9),
         cross.reshape(BPC, N, K, 9),
         cross_norm,
         rel.reshape(BPC, N, K, 9),
         rbf_p, rbf_n, ang], axis=-1)
    h = jnp.tanh(f @ w1 + b1)
    h = jnp.tanh(h @ w2 + b2)
    pooled = jnp.max(h, axis=2)
    return pooled @ w3 + b3


pos_sh = position.reshape(NCORES, BPC, N, 3)
oR_sh = orientation_R.reshape(NCORES, BPC, N, 3, 3)
rep = lambda x: np.broadcast_to(x, (NCORES,) + x.shape).copy()
pm = jax.pmap(per_core, devices=devs)
args = (pos_sh, oR_sh, rep(w1), rep(b1), rep(w2), rep(b2), rep(w3), rep(b3))
out = pm(*args)          # compile + first run
np.asarray(out)          # sync
t0 = time.perf_counter()
NRUN = 3
for _ in range(NRUN):
    out = pm(*args)
out_np = np.asarray(out)
t1 = time.perf_counter()
hw_ns = (t1 - t0) / NRUN * 1e9
np.savez(out_path, out=out_np.reshape(B, N, 1).astype(np.float32),
         hw_ns=np.float64(hw_ns))
'''

LAST_HW_NS = None


def _features_np(position, orientation_R):
    dr = position[:, :, None, :] - position[:, None, :, :]
    dr = dr - BOX * np.round(dr / BOX)
    r2 = np.sum(dr * dr, axis=-1)
    r2 = r2 + np.eye(N, dtype=r2.dtype) * 1e9
    idx = np.argsort(r2, axis=-1, kind="stable")[..., :K]
    nn_r2 = np.take_along_axis(r2, idx, axis=-1)
    NN_R = np.sqrt(nn_r2)[..., None]
    NN_dr = np.take_along_axis(dr, idx[..., None], axis=2)
    u = NN_dr / NN_R
    inv_r = 1.0 / NN_R
    Bb = position.shape[0]
    bidx = np.arange(Bb)[:, None, None]
    Rn = orientation_R[bidx, idx]
    Rp = orientation_R
    dot = np.einsum("bnij,bnklj->bnkil", Rp, Rn)
    elem = Rp[:, :, None, :, None, :] * Rn[:, :, :, None, :, :]
    elem_norm = np.linalg.norm(elem, axis=-1)
    cross = np.cross(Rp[:, :, None, :, :], Rn)
    cross_norm = np.linalg.norm(cross, axis=-1)
    rel = np.einsum("bnji,bnkjl->bnkil", Rp, Rn)
    rbf_p = np.exp(-np.einsum("bnkd,bnid->bnki", u, Rp) ** 2)
    rbf_n = np.exp(-np.einsum("bnkd,bnkid->bnki", u, Rn) ** 2)
    tr = rel[..., 0, 0] + rel[..., 1, 1] + rel[..., 2, 2]
    ang = np.arccos(np.clip((tr - 1.0) * 0.5, -1.0 + 1e-6, 1.0 - 1e-6))[..., None]
    return np.concatenate(
        [NN_R, inv_r, u,
         dot.reshape(Bb, N, K, 9), elem.reshape(Bb, N, K, 27),
         elem_norm.reshape(Bb, N, K, 9), cross.reshape(Bb, N, K, 9),
         cross_norm, rel.reshape(Bb, N, K, 9), rbf_p, rbf_n, ang],
        axis=-1).astype(np.float32)


def _host_fallback(position, orientation_R, w1, b1, w2, b2, w3, b3):
    f = _features_np(position, orientation_R)
    h = np.tanh(f @ w1 + b1)
    h = np.tanh(h @ w2 + b2)
    pooled = h.max(axis=2)
    return (pooled @ w3 + b3).astype(np.float32)


def kernel(position, orientation_R, w1, b1, w2, b2, w3, b3):
    global LAST_HW_NS
    arrs = dict(
        position=np.ascontiguousarray(position, np.float32),
        orientation_R=np.ascontiguousarray(orientation_R, np.float32),
        w1=np.ascontiguousarray(w1, np.float32),
        b1=np.ascontiguousarray(b1, np.float32),
        w2=np.ascontiguousarray(w2, np.float32),
        b2=np.ascontiguousarray(b2, np.float32),
        w3=np.ascontiguousarray(w3, np.float32),
        b3=np.ascontiguousarray(b3, np.float32),
    )
    try:
        with tempfile.TemporaryDirectory() as td:
            inp = os.path.join(td, "in.npz")
            outp = os.path.join(td, "out.npz")
            wrk = os.path.join(td, "worker.py")
            np.savez(inp, **arrs)
            with open(wrk, "w") as f:
                f.write(_DEVICE_WORKER)
            env = dict(os.environ)
            env.pop("JAX_PLATFORMS", None)
            r = subprocess.run(
                [sys.executable, wrk, inp, outp],
                timeout=1500, env=env,
                stdout=subprocess.PIPE, stderr=subprocess.PIPE,
            )
            if r.returncode == 0 and os.path.exists(outp):
                d = np.load(outp)
                LAST_HW_NS = float(d["hw_ns"])
                return d["out"]
            sys.stderr.write(
                "device worker failed rc=%s\n%s\n"
                % (r.returncode, r.stderr.decode()[-2000:])
            )
    except Exception as e:  # noqa: BLE001
        sys.stderr.write("device path error: %r\n" % (e,))
    return _host_fallback(**arrs)


# revision 3
# speedup vs baseline: 143.0048x; 143.0048x over previous
"""Trainium kernel for nn_BaseNeighborNN (B=16, N=1024, K=20).

Sharding: data-parallel over the batch dimension across 8 NeuronCores
(2 snapshots per core); weights replicated. Each core computes all-pairs
periodic distances, top-K neighbor selection, the 78 geometric pair
features, the MLP and the neighbor max-pool locally. Host code only
shards inputs / concatenates outputs.

The device attempt runs in a forked subprocess: the neuron toolchain
aborts the process on an unsupported-graph compile error, and the
subprocess boundary turns that into a recoverable failure (host-numpy
fallback keeps the contract).
"""
import os
import subprocess
import sys
import tempfile

import numpy as np

B, N, K = 16, 1024, 20
BOX = 10.0
NCORES = 8
BPC = B // NCORES

_DEVICE_WORKER = r'''
import sys, time
import numpy as np

inp_path, out_path = sys.argv[1], sys.argv[2]
data = np.load(inp_path)
position = data["position"]; orientation_R = data["orientation_R"]
w1 = data["w1"]; b1 = data["b1"]; w2 = data["w2"]; b2 = data["b2"]
w3 = data["w3"]; b3 = data["b3"]

B, N, K = 16, 1024, 20
BOX = 10.0
NCORES = 8
BPC = B // NCORES

import jax, jax.numpy as jnp

devs = [d for d in jax.devices() if d.platform != "cpu"][:NCORES]
assert len(devs) == NCORES, f"need {NCORES} neuron cores, got {len(devs)}"


def per_core(pos, oR, w1, b1, w2, b2, w3, b3):
    dr = pos[:, :, None, :] - pos[:, None, :, :]
    dr = dr - BOX * jnp.round(dr * (1.0 / BOX))
    r2 = jnp.sum(dr * dr, axis=-1)
    r2 = r2 + jnp.eye(N, dtype=r2.dtype) * 1e9
    neg_r2, idx = jax.lax.top_k(-r2, K)
    NN_R = jnp.sqrt(-neg_r2)[..., None]
    NN_dr = jnp.take_along_axis(dr, idx[..., None], axis=2)
    u = NN_dr / NN_R
    inv_r = 1.0 / NN_R
    bidx = jnp.arange(BPC)[:, None, None]
    Rn = oR[bidx, idx]
    Rp = oR
    dot = jnp.einsum("bnij,bnklj->bnkil", Rp, Rn)
    elem = Rp[:, :, None, :, None, :] * Rn[:, :, :, None, :, :]
    elem_norm = jnp.sqrt(jnp.sum(elem * elem, axis=-1))
    cross = jnp.cross(Rp[:, :, None, :, :], Rn)
    cross_norm = jnp.sqrt(jnp.sum(cross * cross, axis=-1))
    rel = jnp.einsum("bnji,bnkjl->bnkil", Rp, Rn)
    rbf_p = jnp.exp(-jnp.einsum("bnkd,bnid->bnki", u, Rp) ** 2)
    rbf_n = jnp.exp(-jnp.einsum("bnkd,bnkid->bnki", u, Rn) ** 2)
    tr = rel[..., 0, 0] + rel[..., 1, 1] + rel[..., 2, 2]
    t = jnp.clip((tr - 1.0) * 0.5, -1.0 + 1e-6, 1.0 - 1e-6)
    # acos(t) = pi/2 - atan(t/sqrt(1-t^2)); mhlo.acos is unsupported here
    ang = (jnp.pi / 2 - jnp.arctan(t * jax.lax.rsqrt(1.0 - t * t)))[..., None]
    f = jnp.concatenate(
        [NN_R, inv_r, u,
         dot.reshape(BPC, N, K, 9),
         elem.reshape(BPC, N, K, 27),
         elem_norm.reshape(BPC, N, K, 9),
         cross.reshape(BPC, N, K, 9),
         cross_norm,
         rel.reshape(BPC, N, K, 9),
         rbf_p, rbf_n, ang], axis=-1)
    h = jnp.tanh(f @ w1 + b1)
    h = jnp.tanh(h @ w2 + b2)
    pooled = jnp.max(h, axis=2)
    return pooled @ w3 + b3


pos_sh = position.reshape(NCORES, BPC, N, 3)
oR_sh = orientation_R.reshape(NCORES, BPC, N, 3, 3)
rep = lambda x: np.broadcast_to(x, (NCORES,) + x.shape).copy()
pm = jax.pmap(per_core, devices=devs)
args = (pos_sh, oR_sh, rep(w1), rep(b1), rep(w2), rep(b2), rep(w3), rep(b3))
out = pm(*args)          # compile + first run
np.asarray(out)          # sync
t0 = time.perf_counter()
NRUN = 3
for _ in range(NRUN):
    out = pm(*args)
out_np = np.asarray(out)
t1 = time.perf_counter()
hw_ns = (t1 - t0) / NRUN * 1e9
np.savez(out_path, out=out_np.reshape(B, N, 1).astype(np.float32),
         hw_ns=np.float64(hw_ns))
'''

LAST_HW_NS = None


def _features_np(position, orientation_R):
    dr = position[:, :, None, :] - position[:, None, :, :]
    dr = dr - BOX * np.round(dr / BOX)
    r2 = np.sum(dr * dr, axis=-1)
    r2 = r2 + np.eye(N, dtype=r2.dtype) * 1e9
    idx = np.argsort(r2, axis=-1, kind="stable")[..., :K]
    nn_r2 = np.take_along_axis(r2, idx, axis=-1)
    NN_R = np.sqrt(nn_r2)[..., None]
    NN_dr = np.take_along_axis(dr, idx[..., None], axis=2)
    u = NN_dr / NN_R
    inv_r = 1.0 / NN_R
    Bb = position.shape[0]
    bidx = np.arange(Bb)[:, None, None]
    Rn = orientation_R[bidx, idx]
    Rp = orientation_R
    dot = np.einsum("bnij,bnklj->bnkil", Rp, Rn)
    elem = Rp[:, :, None, :, None, :] * Rn[:, :, :, None, :, :]
    elem_norm = np.linalg.norm(elem, axis=-1)
    cross = np.cross(Rp[:, :, None, :, :], Rn)
    cross_norm = np.linalg.norm(cross, axis=-1)
    rel = np.einsum("bnji,bnkjl->bnkil", Rp, Rn)
    rbf_p = np.exp(-np.einsum("bnkd,bnid->bnki", u, Rp) ** 2)
    rbf_n = np.exp(-np.einsum("bnkd,bnkid->bnki", u, Rn) ** 2)
    tr = rel[..., 0, 0] + rel[..., 1, 1] + rel[..., 2, 2]
    ang = np.arccos(np.clip((tr - 1.0) * 0.5, -1.0 + 1e-6, 1.0 - 1e-6))[..., None]
    return np.concatenate(
        [NN_R, inv_r, u,
         dot.reshape(Bb, N, K, 9), elem.reshape(Bb, N, K, 27),
         elem_norm.reshape(Bb, N, K, 9), cross.reshape(Bb, N, K, 9),
         cross_norm, rel.reshape(Bb, N, K, 9), rbf_p, rbf_n, ang],
        axis=-1).astype(np.float32)


def _host_fallback(position, orientation_R, w1, b1, w2, b2, w3, b3):
    f = _features_np(position, orientation_R)
    h = np.tanh(f @ w1 + b1)
    h = np.tanh(h @ w2 + b2)
    pooled = h.max(axis=2)
    return (pooled @ w3 + b3).astype(np.float32)


def kernel(position, orientation_R, w1, b1, w2, b2, w3, b3):
    global LAST_HW_NS
    arrs = dict(
        position=np.ascontiguousarray(position, np.float32),
        orientation_R=np.ascontiguousarray(orientation_R, np.float32),
        w1=np.ascontiguousarray(w1, np.float32),
        b1=np.ascontiguousarray(b1, np.float32),
        w2=np.ascontiguousarray(w2, np.float32),
        b2=np.ascontiguousarray(b2, np.float32),
        w3=np.ascontiguousarray(w3, np.float32),
        b3=np.ascontiguousarray(b3, np.float32),
    )
    try:
        with tempfile.TemporaryDirectory() as td:
            inp = os.path.join(td, "in.npz")
            outp = os.path.join(td, "out.npz")
            wrk = os.path.join(td, "worker.py")
            np.savez(inp, **arrs)
            with open(wrk, "w") as f:
                f.write(_DEVICE_WORKER)
            env = dict(os.environ)
            env.pop("JAX_PLATFORMS", None)
            r = subprocess.run(
                [sys.executable, wrk, inp, outp],
                timeout=1500, env=env,
                stdout=subprocess.PIPE, stderr=subprocess.PIPE,
            )
            if r.returncode == 0 and os.path.exists(outp):
                d = np.load(outp)
                LAST_HW_NS = float(d["hw_ns"])
                return d["out"]
            sys.stderr.write(
                "device worker failed rc=%s\n%s\n"
                % (r.returncode, r.stderr.decode()[-2000:])
            )
    except Exception as e:  # noqa: BLE001
        sys.stderr.write("device path error: %r\n" % (e,))
    return _host_fallback(**arrs)
